# Initial kernel scaffold
#
"""Swin-style window-attention block (nn_Block_25718264168914) for 8x TRN2
NeuronCores. Data-parallel over batch: 4 images per core, no collectives.

Layout strategy per core (see comments inline):
  - activations flow channel-major [C, T] for matmuls (weights used as-stored
    as lhsT), token-major [T, C] for LN/softmax-normalize/residuals.
  - attention uses PE array packing (32x64 tiles for scores, 64x64 for AV).
  - softmax denominators come from a ones-column appended to V.
  - rel-pos bias applied as exp(bias) multiplier (host-precomputed), which
    also zeroes the padding lanes used to keep PSUM fully initialized.
"""

import sys

sys.path.insert(0, "/opt/trn_rl_repo")

import numpy as np
import ml_dtypes

import concourse.bass as bass
import concourse.tile as tile
from concourse import mybir
from concourse.bass_utils import run_bass_kernel_spmd
from concourse.masks import make_identity

# ---------------------------------------------------------------- constants
WS = 7
NH = 8
C = 256
HD = C // NH  # 32
SCALE = HD ** -0.5
EPS = 1e-5
B, H, W = 32, 56, 56
HID = 4 * C  # 1024
N_CORES = 8
BL = B // N_CORES  # images per core
NWS = H // WS  # 8 windows per side
WINS_PER_CORE = BL * NWS * NWS  # 256
CHUNK = 2 * WS * WS  # 98 tokens = 2 windows
CHUNKS_PER_MACRO = 4
MACRO_WINS = 2 * CHUNKS_PER_MACRO  # 8 windows
N_MACRO = WINS_PER_CORE // MACRO_WINS  # 32
NTOK_MACRO = MACRO_WINS * WS * WS  # 392
KPAD = 416  # 392 tokens + 24 zero pad (window padding to 64)

F32 = mybir.dt.float32
BF16 = mybir.dt.bfloat16
AF = mybir.ActivationFunctionType
ALU = mybir.AluOpType


def _rel_pos_index():
    coords = np.stack(
        np.meshgrid(np.arange(WS), np.arange(WS), indexing="ij"), 0
    ).reshape(2, -1)
    rel = (coords[:, :, None] - coords[:, None, :]).transpose(1, 2, 0)
    return (rel[:, :, 0] + WS - 1) * (2 * WS - 1) + (rel[:, :, 1] + WS - 1)


REL_IDX = _rel_pos_index()  # [49, 49] int


# ------------------------------------------------------- drain wait patch
# This walrus build's TPB_CTRL carries at most one sem wait; the TileContext
# tail drain waits on every touched processor. Redistribute the waits across
# single-wait NOPs emitted just before the drain.
def _install_drain_patch():
    import concourse.tile as _tile_mod
    from concourse.vector_clock import ScopedClock as _ScopedClock

    if getattr(_tile_mod.TileContext, "_drain_patch_installed", False):
        return

    def _patched(self, tick_clock, wait_clock):
        nops = [self.nc.sync.nop(nofuse=True) for _ in range(40)]
        drain_inst = self.nc.sync.drain()
        wait_clock.add_sem_waits(
            drain_inst.ins, _ScopedClock({None: tick_clock.global_clock})
        )
        si = drain_inst.ins.sync_info
        waits = list(si.on_wait) if si and si.on_wait else []
        if len(waits) > 1:
            assert len(waits) <= len(nops) + 1
            drain_inst.ins.sync_info = mybir.SyncInfo(
                on_wait=waits[:1], on_update=si.on_update or []
            )
            for nop, wt in zip(nops, waits[1:]):
                nop.ins.sync_info = mybir.SyncInfo(on_wait=[wt], on_update=[])
        self.nc.all_engine_barrier()
        assert self.sems is not None
        popped = self.nc._tile_sem_poison_stack.pop()
        assert popped is self._sem_poison
        self.nc.clear_and_free_semaphores(list(self.sems.allocated().values()))
        self.nc.all_engine_barrier()

    _tile_mod.TileContext._drain_and_barrier = _patched
    _tile_mod.TileContext._drain_patch_installed = True


# This walrus build accepts at most ONE sem wait per instruction. Tile can
# emit several (multi-producer deps). Split: insert single-wait NOPs on the
# same engine immediately before the offending instruction.
_waitnop_counter = [0]


def _split_multi_waits(nc):
    for f in nc.m.functions:
        for bb in f.blocks:
            insts = bb.instructions
            out = []
            changed = False
            for inst in insts:
                si = inst.sync_info
                waits = list(si.on_wait) if si and si.on_wait else []
                if len(waits) > 1:
                    changed = True
                    for wt in waits[:-1]:
                        _waitnop_counter[0] += 1
                        nop = mybir.InstNoOp(
                            name=f"I-waitsplit-{_waitnop_counter[0]}",
                            ins=[],
                            outs=[],
                        )
                        nop.engine = inst.engine
                        nop.sync_info = mybir.SyncInfo(on_wait=[wt], on_update=[])
                        try:
                            nc.register_instruction(nop, overwrite=True)
                        except Exception:
                            pass
                        out.append(nop)
                    inst.sync_info = mybir.SyncInfo(
                        on_wait=[waits[-1]], on_update=si.on_update or []
                    )
                out.append(inst)
            if changed:
                bb.instructions = out


# ------------------------------------------------------------ bass program
def build_program(n_macro=N_MACRO, split_waits=True, stage=99):
    _install_drain_patch()
    nc = bass.Bass()

    d_x = nc.dram_tensor("x", [BL, H, W, C], F32, kind="ExternalInput")
    d_wq = nc.dram_tensor("wq", [128, 2, C], BF16, kind="ExternalInput")
    d_wk = nc.dram_tensor("wk", [128, 2, C], BF16, kind="ExternalInput")
    d_wv = nc.dram_tensor("wv", [128, 2, C], BF16, kind="ExternalInput")
    d_wp = nc.dram_tensor("wp", [128, 2, C], BF16, kind="ExternalInput")
    d_w1 = nc.dram_tensor("w1", [128, 2, HID], BF16, kind="ExternalInput")
    d_w2 = nc.dram_tensor("w2", [128, 8, C], BF16, kind="ExternalInput")
    d_bq = nc.dram_tensor("bq", [128, 2], F32, kind="ExternalInput")
    d_bk = nc.dram_tensor("bk", [128, 2], F32, kind="ExternalInput")
    d_b1 = nc.dram_tensor("b1", [128, 8], F32, kind="ExternalInput")
    d_expb = nc.dram_tensor("expb", [128, 4, 98], BF16, kind="ExternalInput")
    d_out = nc.dram_tensor("out", [BL, H, W, C], F32, kind="ExternalOutput")

    # windowed views: [BL, wr, r, wc, c, ch] for gather/scatter DMA
    xw = d_x.rearrange("b (wr r) (wc c) ch -> b wr r wc c ch", r=WS, c=WS)
    ow = d_out.rearrange("b (wr r) (wc c) ch -> b wr r wc c ch", r=WS, c=WS)
    of = d_out.rearrange("b h w ch -> (b h w) ch")  # flat [12544, 256] dbg view

    from contextlib import ExitStack

    with tile.TileContext(nc) as tc:
        with ExitStack() as ctx:
            resident = ctx.enter_context(tc.tile_pool(name="resident", bufs=1))
            xin_pool = ctx.enter_context(tc.tile_pool(name="xin", bufs=8))
            stats_pool = ctx.enter_context(tc.tile_pool(name="stats", bufs=8))
            xhat_pool = ctx.enter_context(tc.tile_pool(name="xhat", bufs=4))
            hT_pool = ctx.enter_context(tc.tile_pool(name="hT", bufs=2))
            qk_pool = ctx.enter_context(tc.tile_pool(name="qk", bufs=2))
            vaug_pool = ctx.enter_context(tc.tile_pool(name="vaug", bufs=6))
            esf_pool = ctx.enter_context(tc.tile_pool(name="esf", bufs=3))
            es_pool = ctx.enter_context(tc.tile_pool(name="es", bufs=4))
            attn_pool = ctx.enter_context(tc.tile_pool(name="attn", bufs=10))
            attnT_pool = ctx.enter_context(tc.tile_pool(name="attnT", bufs=2))
            x2_pool = ctx.enter_context(tc.tile_pool(name="x2", bufs=8))
            h2T_pool = ctx.enter_context(tc.tile_pool(name="h2T", bufs=2))
            gT_pool = ctx.enter_context(tc.tile_pool(name="gT", bufs=2))
            otm_pool = ctx.enter_context(tc.tile_pool(name="otm", bufs=4))
            ps_tr = ctx.enter_context(tc.tile_pool(name="ps_tr", bufs=1, space="PSUM"))
            ps_big = ctx.enter_context(tc.tile_pool(name="ps_big", bufs=2, space="PSUM"))
            ps_sc = ctx.enter_context(tc.tile_pool(name="ps_sc", bufs=1, space="PSUM"))
            ps_av_pool = ctx.enter_context(tc.tile_pool(name="ps_av", bufs=1, space="PSUM"))
            ps_sm = ctx.enter_context(tc.tile_pool(name="ps_sm", bufs=2, space="PSUM"))
            # ---------------- residents
            wq_sb = resident.tile([128, 2, C], BF16)
            nc.sync.dma_start(wq_sb, d_wq[:])
            wk_sb = resident.tile([128, 2, C], BF16)
            nc.sync.dma_start(wk_sb, d_wk[:])
            wv_sb = resident.tile([128, 2, C], BF16)
            nc.sync.dma_start(wv_sb, d_wv[:])
            wp_sb = resident.tile([128, 2, C], BF16)
            nc.sync.dma_start(wp_sb, d_wp[:])
            w1_sb = resident.tile([128, 2, HID], BF16)
            nc.sync.dma_start(w1_sb, d_w1[:])
            w2_sb = resident.tile([128, 8, C], BF16)
            nc.sync.dma_start(w2_sb, d_w2[:])
            bq_sb = resident.tile([128, 2], F32)
            nc.sync.dma_start(bq_sb, d_bq[:])
            bk_sb = resident.tile([128, 2], F32)
            nc.sync.dma_start(bk_sb, d_bk[:])
            b1_sb = resident.tile([128, 8], F32)
            nc.sync.dma_start(b1_sb, d_b1[:])
            expb_sb = resident.tile([128, 4, 98], BF16)
            nc.sync.dma_start(expb_sb, d_expb[:])
            ident = resident.tile([128, 128], BF16)
            make_identity(nc, ident)
            eps_sb = resident.tile([128, 1], F32)
            nc.vector.memset(eps_sb, EPS)

            for mt in range(n_macro):
                # windows mt*8 .. mt*8+7; chunks of 2 windows
                hT = hT_pool.tile([128, 2, KPAD], BF16)
                nc.vector.memset(hT[:, :, NTOK_MACRO:KPAD], 0.0)
                x_chunks = []
                for ck in range(CHUNKS_PER_MACRO):
                    g = mt * MACRO_WINS + ck * 2  # first window of pair
                    b_i = g // (NWS * NWS)
                    wr = (g % (NWS * NWS)) // NWS
                    wc = g % NWS
                    # pitch 264 != 256 so the window-gather AP cannot be
                    # flat-merged across the partition boundary
                    x_tile = xin_pool.tile([98, C + 8], F32, tag="x_tm")
                    x_tm = x_tile[:, 0:C]
                    for w01 in range(2):
                        src = xw[b_i, wr, :, wc + w01, :, :]
                        nc.sync.dma_start(
                            x_tm[w01 * 49 : (w01 + 1) * 49, :], src
                        )
                    x_chunks.append(x_tm)
                    if stage == 1:
                        for w01 in range(2):
                            dst = ow[b_i, wr, :, wc + w01, :, :]
                            nc.sync.dma_start(
                                dst, x_tm[w01 * 49 : (w01 + 1) * 49, :]
                            )
                        continue

                    # ---- LN1 (token-major)
                    st = stats_pool.tile([98, 6], F32, tag="st")
                    nc.vector.bn_stats(st, x_tm)
                    mv = stats_pool.tile([98, 2], F32, tag="mv")
                    nc.vector.bn_aggr(mv, st)
                    sd = stats_pool.tile([98, 1], F32, tag="sd")
                    nc.scalar.activation(sd, mv[:, 1:2], AF.Sqrt, bias=eps_sb[:98])
                    rstd = stats_pool.tile([98, 1], F32, tag="rstd")
                    nc.vector.reciprocal(rstd, sd)
                    xh = xhat_pool.tile([98, C], BF16, tag="xh")
                    nc.vector.tensor_scalar(
                        xh, x_tm, mv[:, 0:1], rstd, ALU.subtract, ALU.mult
                    )
                    # ---- transpose to channel-major
                    for kc in range(2):
                        pst = ps_tr.tile([128, 98], BF16, tag="pst")
                        nc.tensor.transpose(
                            pst, xh[:, kc * 128 : (kc + 1) * 128], ident[:98, :98]
                        )
                        nc.any.tensor_copy(
                            hT[:, kc, ck * 98 : (ck + 1) * 98], pst
                        )

                if stage == 1:
                    continue
                if stage == 2:
                    for kc in range(2):
                        cvt = otm_pool.tile([128, C + 8], F32, tag="o_tm")
                        nc.any.tensor_copy(cvt[:, 0:C], hT[:, kc, 0:C])
                        nc.sync.dma_start(
                            of[(mt * 2 + kc) * 128 : (mt * 2 + kc + 1) * 128, :],
                            cvt[:, 0:C],
                        )
                    continue

                # ---- q, k (channel-major) ----
                qT = qk_pool.tile([128, 2, NTOK_MACRO], BF16, tag="qT")
                kT = qk_pool.tile([128, 2, KPAD], BF16, tag="kT")
                nc.vector.memset(kT[:, :, NTOK_MACRO:KPAD], 0.0)
                for mc in range(2):
                    psq = ps_big.tile([128, NTOK_MACRO], F32, tag="ps_big")
                    for kc in range(2):
                        nc.tensor.matmul(
                            psq,
                            lhsT=wq_sb[:, kc, mc * 128 : (mc + 1) * 128],
                            rhs=hT[:, kc, 0:NTOK_MACRO],
                            start=(kc == 0),
                            stop=(kc == 1),
                        )
                    nc.scalar.activation(
                        qT[:, mc, :], psq, AF.Identity, bias=bq_sb[:, mc : mc + 1]
                    )
                    psk = ps_big.tile([128, NTOK_MACRO], F32, tag="ps_big")
                    for kc in range(2):
                        nc.tensor.matmul(
                            psk,
                            lhsT=wk_sb[:, kc, mc * 128 : (mc + 1) * 128],
                            rhs=hT[:, kc, 0:NTOK_MACRO],
                            start=(kc == 0),
                            stop=(kc == 1),
                        )
                    nc.scalar.activation(
                        kT[:, mc, 0:NTOK_MACRO],
                        psk,
                        AF.Identity,
                        bias=bk_sb[:, mc : mc + 1],
                    )

                if stage == 3:
                    cvt = otm_pool.tile([128, C + 8], F32, tag="o_tm")
                    nc.any.tensor_copy(cvt[:, 0:C], qT[:, 0, 0:C])
                    nc.any.tensor_copy(cvt[:, 0:C], kT[:, 0, 0:C])
                    nc.sync.dma_start(
                        of[mt * 128 : (mt + 1) * 128, :], cvt[:, 0:C]
                    )
                    continue

                # ---- v (token-major, with ones column) ----
                vaugs = []
                for ck in range(CHUNKS_PER_MACRO):
                    psv = ps_sm.tile([128, C], F32, tag="ps_sm")
                    for w01 in range(2):
                        for kc in range(2):
                            nc.tensor.matmul(
                                psv[w01 * 64 : w01 * 64 + 64, :],
                                lhsT=hT[
                                    :, kc, (ck * 2 + w01) * 49 : (ck * 2 + w01) * 49 + 64
                                ],
                                rhs=wv_sb[:, kc, :],
                                start=(kc == 0),
                                stop=(kc == 1),
                            )
                    va = vaug_pool.tile([128, NH, HD + 1], BF16, tag="va")
                    nc.any.tensor_copy(
                        va[:, :, 0:HD],
                        psv.rearrange("p (h d) -> p h d", h=NH),
                    )
                    nc.vector.memset(va[:, :, HD : HD + 1], 1.0)
                    vaugs.append(va)

                if stage == 4:
                    cvt = otm_pool.tile([128, C + 8], F32, tag="o_tm")
                    nc.any.tensor_copy(
                        cvt[:, 0:C].rearrange("p (h d) -> p h d", h=NH),
                        vaugs[0][:, :, 0:HD],
                    )
                    nc.sync.dma_start(
                        of[mt * 128 : (mt + 1) * 128, :], cvt[:, 0:C]
                    )
                    continue

                # ---- attention per chunk ----
                # PSUM bank rule: concurrent PE row-groups must hit different
                # banks. Scores use one bank per row-group pair (r, r+2 share
                # a slot serially); AV windows rotate through one bank.
                attn_chunks = []
                for ck in range(CHUNKS_PER_MACRO):
                    # scores psum per row-group r: [j(2x64), g*49+i], heads
                    # h = r + 4g live at partitions r*32, ctile g
                    # es block b = 2r+g holds head h = r+4g; log-bias is
                    # preloaded into PSUM (identity matmul) so exp() emits
                    # the finished softmax numerators, pad keys ~ exp(-87)=0
                    es = es_pool.tile([128, 8, 49], BF16, tag="es")
                    for r in range(4):
                        pss = ps_sc.tile(
                            [128, 2 * 49], F32, tag=f"sc{r % 2}",
                            name=f"pss{r}",
                        )
                        nc.tensor.matmul(
                            pss, lhsT=ident, rhs=expb_sb[:, r, :],
                            start=True, stop=True,
                        )
                        for w01 in range(2):
                            t0 = (ck * 2 + w01) * 49
                            for g in range(2):
                                nc.tensor.matmul(
                                    pss[
                                        w01 * 64 : w01 * 64 + 64,
                                        g * 49 : g * 49 + 49,
                                    ],
                                    lhsT=kT[r * 32 : r * 32 + 32, g, t0 : t0 + 64],
                                    rhs=qT[r * 32 : r * 32 + 32, g, t0 : t0 + 49],
                                    start=False,
                                    stop=True,
                                    tile_position=(r * 32, w01 * 64),
                                    skip_group_check=True,
                                )
                        nc.scalar.activation(es[:, 2 * r:2 * r + 2, :], pss,
                                             AF.Exp)

                    if stage == 5:
                        cvt = otm_pool.tile([128, C + 8], F32, tag="o_tm")
                        nc.any.tensor_copy(cvt[:, 0:C], es.rearrange('p a b -> p (a b)')[:, 0:C])
                        nc.sync.dma_start(
                            of[(mt * 4 + ck) * 98 : (mt * 4 + ck) * 98 + 128, :],
                            cvt[:, 0:C],
                        )
                        continue

                    # AV per window: psum[i(49), h*33+d]; K=64 (es zero rows
                    # kill the pad keys). One bank, windows serialized.
                    va = vaugs[ck]
                    for w01 in range(2):
                        jb = w01 * 64
                        psav = ps_av_pool.tile(
                            [64, NH * (HD + 1)], F32, tag="ps_av", name="psav"
                        )
                        for h in range(NH):
                            nc.tensor.matmul(
                                psav[0:49, h * 33 : h * 33 + 33],
                                lhsT=es[jb : jb + 64,
                                        2 * (h % 4) + h // 4, :],
                                rhs=va[jb : jb + 64, h, :],
                                start=True,
                                stop=True,
                                tile_position=(jb, 0),
                            )
                        # normalize + evict (token-major, all rows valid)
                        psav_v = psav.rearrange("p (h e) -> p h e", h=NH)
                        rd = stats_pool.tile([49, NH], F32, tag="rd")
                        nc.vector.reciprocal(rd, psav_v[0:49, :, HD])
                        a_tm = attn_pool.tile([49, NH, HD], BF16, tag="a_tm")
                        nc.vector.tensor_tensor(
                            a_tm,
                            psav_v[0:49, :, 0:HD],
                            rd[:, :, None].to_broadcast([49, NH, HD]),
                            ALU.mult,
                        )
                        attn_chunks.append(a_tm)

                if stage == 5 or stage == 45:
                    continue
                if stage == 6:
                    cvt = otm_pool.tile([128, C + 8], F32, tag="o_tm")
                    nc.any.tensor_copy(
                        cvt[:49, 0:C].rearrange("p (h d) -> p h d", h=NH),
                        attn_chunks[0],
                    )
                    nc.sync.dma_start(
                        of[mt * 128 : mt * 128 + 49, :], cvt[:49, 0:C]
                    )
                    continue

                # ---- transpose attn to channel-major ----
                aT = attnT_pool.tile([128, 2, NTOK_MACRO], BF16, tag="aT")
                for wk_i in range(2 * CHUNKS_PER_MACRO):
                    a_flat = attn_chunks[wk_i].rearrange("p h d -> p (h d)")
                    for kc in range(2):
                        psat = ps_tr.tile([128, 49], BF16, tag="pst")
                        nc.tensor.transpose(
                            psat,
                            a_flat[:, kc * 128 : (kc + 1) * 128],
                            ident[:49, :49],
                        )
                        nc.any.tensor_copy(
                            aT[:, kc, wk_i * 49 : (wk_i + 1) * 49], psat
                        )

                # ---- proj (token-major out) + residual + LN2 + transpose ----
                h2T = h2T_pool.tile([128, 2, NTOK_MACRO], BF16, tag="h2T")
                x2_chunks = []
                for ck in range(CHUNKS_PER_MACRO):
                    psp = ps_sm.tile([98, C], F32, tag="ps_sm")
                    for kc in range(2):
                        nc.tensor.matmul(
                            psp,
                            lhsT=aT[:, kc, ck * 98 : (ck + 1) * 98],
                            rhs=wp_sb[:, kc, :],
                            start=(kc == 0),
                            stop=(kc == 1),
                        )
                    x2 = x2_pool.tile([98, C], F32, tag="x2")
                    nc.vector.tensor_tensor(x2, psp, x_chunks[ck], ALU.add)
                    x2_chunks.append(x2)
                    # LN2
                    st2 = stats_pool.tile([98, 6], F32, tag="st2")
                    nc.vector.bn_stats(st2, x2)
                    mv2 = stats_pool.tile([98, 2], F32, tag="mv2")
                    nc.vector.bn_aggr(mv2, st2)
                    sd2 = stats_pool.tile([98, 1], F32, tag="sd2")
                    nc.scalar.activation(sd2, mv2[:, 1:2], AF.Sqrt, bias=eps_sb[:98])
                    rstd2 = stats_pool.tile([98, 1], F32, tag="rstd2")
                    nc.vector.reciprocal(rstd2, sd2)
                    xh2 = xhat_pool.tile([98, C], BF16, tag="xh2")
                    nc.vector.tensor_scalar(
                        xh2, x2, mv2[:, 0:1], rstd2, ALU.subtract, ALU.mult
                    )
                    for kc in range(2):
                        pst2 = ps_tr.tile([128, 98], BF16, tag="pst")
                        nc.tensor.transpose(
                            pst2, xh2[:, kc * 128 : (kc + 1) * 128], ident[:98, :98]
                        )
                        nc.any.tensor_copy(
                            h2T[:, kc, ck * 98 : (ck + 1) * 98], pst2
                        )

                if stage == 7:
                    for ck in range(CHUNKS_PER_MACRO):
                        g = mt * MACRO_WINS + ck * 2
                        b_i = g // (NWS * NWS)
                        wr = (g % (NWS * NWS)) // NWS
                        wc = g % NWS
                        for w01 in range(2):
                            dst = ow[b_i, wr, :, wc + w01, :, :]
                            nc.sync.dma_start(
                                dst,
                                x2_chunks[ck][w01 * 49 : (w01 + 1) * 49, :],
                            )
                    continue

                # ---- fc1 + gelu (channel-major) ----
                gT = gT_pool.tile([128, 8, NTOK_MACRO], BF16, tag="gT")
                for mc in range(8):
                    psf = ps_big.tile([128, NTOK_MACRO], F32, tag="ps_big")
                    for kc in range(2):
                        nc.tensor.matmul(
                            psf,
                            lhsT=w1_sb[:, kc, mc * 128 : (mc + 1) * 128],
                            rhs=h2T[:, kc, :],
                            start=(kc == 0),
                            stop=(kc == 1),
                        )
                    nc.scalar.activation(
                        gT[:, mc, :], psf, AF.Gelu, bias=b1_sb[:, mc : mc + 1]
                    )

                # ---- fc2 (token-major out) + residual + store ----
                for ck in range(CHUNKS_PER_MACRO):
                    ps2 = ps_sm.tile([98, C], F32, tag="ps_sm")
                    for hc in range(8):
                        nc.tensor.matmul(
                            ps2,
                            lhsT=gT[:, hc, ck * 98 : (ck + 1) * 98],
                            rhs=w2_sb[:, hc, :],
                            start=(hc == 0),
                            stop=(hc == 7),
                        )
                    o_tile = otm_pool.tile([98, C + 8], F32, tag="o_tm")
                    o_tm = o_tile[:, 0:C]
                    nc.vector.tensor_tensor(o_tm, ps2, x2_chunks[ck], ALU.add)
                    g = mt * MACRO_WINS + ck * 2
                    b_i = g // (NWS * NWS)
                    wr = (g % (NWS * NWS)) // NWS
                    wc = g % NWS
                    for w01 in range(2):
                        dst = ow[b_i, wr, :, wc + w01, :, :]
                        nc.sync.dma_start(
                            dst, o_tm[w01 * 49 : (w01 + 1) * 49, :]
                        )

    if split_waits:
        _split_multi_waits(nc)
    return nc


# ------------------------------------------------------------- host wrapper
_PROGRAM_CACHE = {}


def _prep_weights(norm1_g, norm1_b, qkv_w, qkv_b, bias_table, proj_w, proj_b,
                  norm2_g, norm2_b, fc1_w, fc1_b, fc2_w, fc2_b):
    f32 = np.float32
    bf16 = ml_dtypes.bfloat16
    # fold LN1 affine into qkv weights
    wqkv = (norm1_g[:, None] * qkv_w).astype(f32)  # [C, 3C]
    bqkv = (norm1_b @ qkv_w + qkv_b).astype(f32)  # [3C]
    wq = wqkv[:, 0:C] * SCALE
    bq = bqkv[0:C] * SCALE
    wk = wqkv[:, C : 2 * C]
    bk = bqkv[C : 2 * C]
    wv = wqkv[:, 2 * C : 3 * C]
    bv = bqkv[2 * C : 3 * C]
    # fold LN2 affine into fc1
    w1 = (norm2_g[:, None] * fc1_w).astype(f32)  # [C, HID]
    b1 = (norm2_b @ fc1_w + fc1_b).astype(f32)  # [HID]

    def kpart(w):  # [K, O] -> [128, K//128, O]
        k, o = w.shape
        return np.ascontiguousarray(
            w.reshape(k // 128, 128, o).transpose(1, 0, 2)
        )

    arrs = {
        "wq": kpart(wq).astype(bf16),
        "wk": kpart(wk).astype(bf16),
        "wv": kpart(wv).astype(bf16),
        "wp": kpart(proj_w.astype(f32)).astype(bf16),
        "w1": kpart(w1).astype(bf16),
        "w2": kpart(fc2_w.astype(f32)).astype(bf16),
        "bq": np.ascontiguousarray(bq.reshape(2, 128).T).astype(f32),
        "bk": np.ascontiguousarray(bk.reshape(2, 128).T).astype(f32),
        "b1": np.ascontiguousarray(b1.reshape(8, 128).T).astype(f32),
    }
    # log-domain rel-pos bias, PSUM-preloaded: lb[jrow, r, g*49+i] =
    # bias[i, j, h=r+4g]; -87 at pad key rows so exp() zeroes them
    bias_full = bias_table[REL_IDX]  # [i, j, NH]
    lb = np.full((128, 4, 98), -87.0, dtype=f32)
    for h in range(NH):
        r, g = h % 4, h // 4
        bj = bias_full[:, :, h].T.astype(f32)  # [j, i]
        lb[0:49, r, g * 49:(g + 1) * 49] = bj
        lb[64:113, r, g * 49:(g + 1) * 49] = bj
    arrs["expb"] = lb.astype(ml_dtypes.bfloat16)

    # token-major biases (v, proj, fc2) must be zero for this build
    for name, v in (("bv", bv), ("bp", proj_b), ("b2", fc2_b)):
        assert np.abs(v).max() < 1e-30, f"nonzero {name} not supported yet"
    return arrs


def kernel(**inputs):
    x = np.asarray(inputs["x"], dtype=np.float32)
    prep = _prep_weights(
        np.asarray(inputs["norm1_g"], np.float32),
        np.asarray(inputs["norm1_b"], np.float32),
        np.asarray(inputs["qkv_w"], np.float32),
        np.asarray(inputs["qkv_b"], np.float32),
        np.asarray(inputs["bias_table"], np.float32),
        np.asarray(inputs["proj_w"], np.float32),
        np.asarray(inputs["proj_b"], np.float32),
        np.asarray(inputs["norm2_g"], np.float32),
        np.asarray(inputs["norm2_b"], np.float32),
        np.asarray(inputs["fc1_w"], np.float32),
        np.asarray(inputs["fc1_b"], np.float32),
        np.asarray(inputs["fc2_w"], np.float32),
        np.asarray(inputs["fc2_b"], np.float32),
    )

    if "nc" not in _PROGRAM_CACHE:
        _PROGRAM_CACHE["nc"] = build_program()
    nc = _PROGRAM_CACHE["nc"]

    in_maps = []
    for c in range(N_CORES):
        m = dict(prep)
        m["x"] = np.ascontiguousarray(x[c * BL : (c + 1) * BL])
        in_maps.append(m)

    res = run_bass_kernel_spmd(nc, in_maps, core_ids=list(range(N_CORES)))
    out = np.concatenate([res.results[c]["out"] for c in range(N_CORES)], axis=0)
    return out.astype(np.float32)



# revision 1
# speedup vs baseline: 1.4404x; 1.4404x over previous
"""Swin-style window-attention block (nn_Block_25718264168914) for 8x TRN2
NeuronCores. Data-parallel over batch: 4 images per core, no collectives.

Layout strategy per core (see comments inline):
  - activations flow channel-major [C, T] for matmuls (weights used as-stored
    as lhsT), token-major [T, C] for LN/softmax-normalize/residuals.
  - attention uses PE array packing (32x64 tiles for scores, 64x64 for AV).
  - softmax denominators come from a ones-column appended to V.
  - rel-pos bias applied as exp(bias) multiplier (host-precomputed), which
    also zeroes the padding lanes used to keep PSUM fully initialized.
"""

import sys

sys.path.insert(0, "/opt/trn_rl_repo")

import numpy as np
import ml_dtypes

import concourse.bass as bass
import concourse.tile as tile
from concourse import mybir
from concourse.bass_utils import run_bass_kernel_spmd
from concourse.masks import make_identity

# ---------------------------------------------------------------- constants
WS = 7
NH = 8
C = 256
HD = C // NH  # 32
SCALE = HD ** -0.5
EPS = 1e-5
B, H, W = 32, 56, 56
HID = 4 * C  # 1024
N_CORES = 8
BL = B // N_CORES  # images per core
NWS = H // WS  # 8 windows per side
WINS_PER_CORE = BL * NWS * NWS  # 256
CHUNK = 2 * WS * WS  # 98 tokens = 2 windows
CHUNKS_PER_MACRO = 4
MACRO_WINS = 2 * CHUNKS_PER_MACRO  # 8 windows
N_MACRO = WINS_PER_CORE // MACRO_WINS  # 32
NTOK_MACRO = MACRO_WINS * WS * WS  # 392
KPAD = 416  # 392 tokens + 24 zero pad (window padding to 64)

F32 = mybir.dt.float32
BF16 = mybir.dt.bfloat16
AF = mybir.ActivationFunctionType
ALU = mybir.AluOpType


def _rel_pos_index():
    coords = np.stack(
        np.meshgrid(np.arange(WS), np.arange(WS), indexing="ij"), 0
    ).reshape(2, -1)
    rel = (coords[:, :, None] - coords[:, None, :]).transpose(1, 2, 0)
    return (rel[:, :, 0] + WS - 1) * (2 * WS - 1) + (rel[:, :, 1] + WS - 1)


REL_IDX = _rel_pos_index()  # [49, 49] int


# ------------------------------------------------------- drain wait patch
# This walrus build's TPB_CTRL carries at most one sem wait; the TileContext
# tail drain waits on every touched processor. Redistribute the waits across
# single-wait NOPs emitted just before the drain.
def _install_drain_patch():
    import concourse.tile as _tile_mod
    from concourse.vector_clock import ScopedClock as _ScopedClock

    if getattr(_tile_mod.TileContext, "_drain_patch_installed", False):
        return

    def _patched(self, tick_clock, wait_clock):
        nops = [self.nc.sync.nop(nofuse=True) for _ in range(40)]
        drain_inst = self.nc.sync.drain()
        wait_clock.add_sem_waits(
            drain_inst.ins, _ScopedClock({None: tick_clock.global_clock})
        )
        si = drain_inst.ins.sync_info
        waits = list(si.on_wait) if si and si.on_wait else []
        if len(waits) > 1:
            assert len(waits) <= len(nops) + 1
            drain_inst.ins.sync_info = mybir.SyncInfo(
                on_wait=waits[:1], on_update=si.on_update or []
            )
            for nop, wt in zip(nops, waits[1:]):
                nop.ins.sync_info = mybir.SyncInfo(on_wait=[wt], on_update=[])
        self.nc.all_engine_barrier()
        assert self.sems is not None
        popped = self.nc._tile_sem_poison_stack.pop()
        assert popped is self._sem_poison
        self.nc.clear_and_free_semaphores(list(self.sems.allocated().values()))
        self.nc.all_engine_barrier()

    _tile_mod.TileContext._drain_and_barrier = _patched
    _tile_mod.TileContext._drain_patch_installed = True


# This walrus build accepts at most ONE sem wait per instruction. Tile can
# emit several (multi-producer deps). Split: insert single-wait NOPs on the
# same engine immediately before the offending instruction.
_waitnop_counter = [0]


def _split_multi_waits(nc):
    for f in nc.m.functions:
        for bb in f.blocks:
            insts = bb.instructions
            out = []
            changed = False
            for inst in insts:
                si = inst.sync_info
                waits = list(si.on_wait) if si and si.on_wait else []
                if len(waits) > 1:
                    changed = True
                    for wt in waits[:-1]:
                        _waitnop_counter[0] += 1
                        nop = mybir.InstNoOp(
                            name=f"I-waitsplit-{_waitnop_counter[0]}",
                            ins=[],
                            outs=[],
                        )
                        nop.engine = inst.engine
                        nop.sync_info = mybir.SyncInfo(on_wait=[wt], on_update=[])
                        try:
                            nc.register_instruction(nop, overwrite=True)
                        except Exception:
                            pass
                        out.append(nop)
                    inst.sync_info = mybir.SyncInfo(
                        on_wait=[waits[-1]], on_update=si.on_update or []
                    )
                out.append(inst)
            if changed:
                bb.instructions = out


# ------------------------------------------------------------ bass program
def build_program(n_macro=N_MACRO, split_waits=True, stage=99):
    _install_drain_patch()
    nc = bass.Bass()

    d_x = nc.dram_tensor("x", [BL, H, W, C], F32, kind="ExternalInput")
    d_wq = nc.dram_tensor("wq", [128, 2, C], BF16, kind="ExternalInput")
    d_wk = nc.dram_tensor("wk", [128, 2, C], BF16, kind="ExternalInput")
    d_wv = nc.dram_tensor("wv", [128, 2, C], BF16, kind="ExternalInput")
    d_wp = nc.dram_tensor("wp", [128, 2, C], BF16, kind="ExternalInput")
    d_w1 = nc.dram_tensor("w1", [128, 2, HID], BF16, kind="ExternalInput")
    d_w2 = nc.dram_tensor("w2", [128, 8, C], BF16, kind="ExternalInput")
    d_bq = nc.dram_tensor("bq", [128, 2], F32, kind="ExternalInput")
    d_bk = nc.dram_tensor("bk", [128, 2], F32, kind="ExternalInput")
    d_b1 = nc.dram_tensor("b1", [128, 8], F32, kind="ExternalInput")
    d_expb = nc.dram_tensor("expb", [128, 4, 98], BF16, kind="ExternalInput")
    d_out = nc.dram_tensor("out", [BL, H, W, C], F32, kind="ExternalOutput")

    # windowed views: [BL, wr, r, wc, c, ch] for gather/scatter DMA
    xw = d_x.rearrange("b (wr r) (wc c) ch -> b wr r wc c ch", r=WS, c=WS)
    ow = d_out.rearrange("b (wr r) (wc c) ch -> b wr r wc c ch", r=WS, c=WS)
    of = d_out.rearrange("b h w ch -> (b h w) ch")  # flat [12544, 256] dbg view

    from contextlib import ExitStack

    with tile.TileContext(nc) as tc:
        with ExitStack() as ctx:
            resident = ctx.enter_context(tc.tile_pool(name="resident", bufs=1))
            xin_pool = ctx.enter_context(tc.tile_pool(name="xin", bufs=8))
            stats_pool = ctx.enter_context(tc.tile_pool(name="stats", bufs=8))
            xhat_pool = ctx.enter_context(tc.tile_pool(name="xhat", bufs=4))
            hT_pool = ctx.enter_context(tc.tile_pool(name="hT", bufs=2))
            qk_pool = ctx.enter_context(tc.tile_pool(name="qk", bufs=2))
            vaug_pool = ctx.enter_context(tc.tile_pool(name="vaug", bufs=6))
            esf_pool = ctx.enter_context(tc.tile_pool(name="esf", bufs=3))
            es_pool = ctx.enter_context(tc.tile_pool(name="es", bufs=4))
            attn_pool = ctx.enter_context(tc.tile_pool(name="attn", bufs=10))
            attnT_pool = ctx.enter_context(tc.tile_pool(name="attnT", bufs=2))
            x2_pool = ctx.enter_context(tc.tile_pool(name="x2", bufs=8))
            h2T_pool = ctx.enter_context(tc.tile_pool(name="h2T", bufs=2))
            gT_pool = ctx.enter_context(tc.tile_pool(name="gT", bufs=2))
            otm_pool = ctx.enter_context(tc.tile_pool(name="otm", bufs=4))
            ps_tr = ctx.enter_context(tc.tile_pool(name="ps_tr", bufs=1, space="PSUM"))
            ps_big = ctx.enter_context(tc.tile_pool(name="ps_big", bufs=2, space="PSUM"))
            ps_sc = ctx.enter_context(tc.tile_pool(name="ps_sc", bufs=1, space="PSUM"))
            ps_av_pool = ctx.enter_context(tc.tile_pool(name="ps_av", bufs=1, space="PSUM"))
            ps_sm = ctx.enter_context(tc.tile_pool(name="ps_sm", bufs=2, space="PSUM"))
            # ---------------- residents
            wq_sb = resident.tile([128, 2, C], BF16)
            nc.sync.dma_start(wq_sb, d_wq[:])
            wk_sb = resident.tile([128, 2, C], BF16)
            nc.sync.dma_start(wk_sb, d_wk[:])
            wv_sb = resident.tile([128, 2, C], BF16)
            nc.sync.dma_start(wv_sb, d_wv[:])
            wp_sb = resident.tile([128, 2, C], BF16)
            nc.sync.dma_start(wp_sb, d_wp[:])
            w1_sb = resident.tile([128, 2, HID], BF16)
            nc.sync.dma_start(w1_sb, d_w1[:])
            w2_sb = resident.tile([128, 8, C], BF16)
            nc.sync.dma_start(w2_sb, d_w2[:])
            bq_sb = resident.tile([128, 2], F32)
            nc.sync.dma_start(bq_sb, d_bq[:])
            bk_sb = resident.tile([128, 2], F32)
            nc.sync.dma_start(bk_sb, d_bk[:])
            b1_sb = resident.tile([128, 8], F32)
            nc.sync.dma_start(b1_sb, d_b1[:])
            expb_sb = resident.tile([128, 4, 98], BF16)
            nc.sync.dma_start(expb_sb, d_expb[:])
            ident = resident.tile([128, 128], BF16)
            make_identity(nc, ident)
            eps_sb = resident.tile([128, 1], F32)
            nc.vector.memset(eps_sb, EPS)

            for mt in range(n_macro):
                # windows mt*8 .. mt*8+7; chunks of 2 windows
                hT = hT_pool.tile([128, 2, KPAD], BF16)
                nc.vector.memset(hT[:, :, NTOK_MACRO:KPAD], 0.0)
                x_chunks = []
                for ck in range(CHUNKS_PER_MACRO):
                    g = mt * MACRO_WINS + ck * 2  # first window of pair
                    b_i = g // (NWS * NWS)
                    wr = (g % (NWS * NWS)) // NWS
                    wc = g % NWS
                    # pitch 264 != 256 so the window-gather AP cannot be
                    # flat-merged across the partition boundary
                    x_tile = xin_pool.tile([98, C + 8], F32, tag="x_tm")
                    x_tm = x_tile[:, 0:C]
                    for w01 in range(2):
                        src = xw[b_i, wr, :, wc + w01, :, :]
                        nc.sync.dma_start(
                            x_tm[w01 * 49 : (w01 + 1) * 49, :], src
                        )
                    x_chunks.append(x_tm)
                    if stage == 1:
                        for w01 in range(2):
                            dst = ow[b_i, wr, :, wc + w01, :, :]
                            nc.sync.dma_start(
                                dst, x_tm[w01 * 49 : (w01 + 1) * 49, :]
                            )
                        continue

                    # ---- LN1 (token-major)
                    st = stats_pool.tile([98, 6], F32, tag="st")
                    nc.vector.bn_stats(st, x_tm)
                    mv = stats_pool.tile([98, 2], F32, tag="mv")
                    nc.vector.bn_aggr(mv, st)
                    sd = stats_pool.tile([98, 1], F32, tag="sd")
                    nc.scalar.activation(sd, mv[:, 1:2], AF.Sqrt, bias=eps_sb[:98])
                    rstd = stats_pool.tile([98, 1], F32, tag="rstd")
                    nc.vector.reciprocal(rstd, sd)
                    xh = xhat_pool.tile([98, C], BF16, tag="xh")
                    nc.vector.tensor_scalar(
                        xh, x_tm, mv[:, 0:1], rstd, ALU.subtract, ALU.mult
                    )
                    # ---- transpose to channel-major
                    for kc in range(2):
                        pst = ps_tr.tile([128, 98], BF16, tag="pst")
                        nc.tensor.transpose(
                            pst, xh[:, kc * 128 : (kc + 1) * 128], ident[:98, :98]
                        )
                        nc.any.tensor_copy(
                            hT[:, kc, ck * 98 : (ck + 1) * 98], pst
                        )

                if stage == 1:
                    continue
                if stage == 2:
                    for kc in range(2):
                        cvt = otm_pool.tile([128, C + 8], F32, tag="o_tm")
                        nc.any.tensor_copy(cvt[:, 0:C], hT[:, kc, 0:C])
                        nc.sync.dma_start(
                            of[(mt * 2 + kc) * 128 : (mt * 2 + kc + 1) * 128, :],
                            cvt[:, 0:C],
                        )
                    continue

                # ---- q, k (channel-major) ----
                qT = qk_pool.tile([128, 2, NTOK_MACRO], BF16, tag="qT")
                kT = qk_pool.tile([128, 2, KPAD], BF16, tag="kT")
                nc.vector.memset(kT[:, :, NTOK_MACRO:KPAD], 0.0)
                for mc in range(2):
                    psq = ps_big.tile([128, NTOK_MACRO], F32, tag="ps_big")
                    for kc in range(2):
                        nc.tensor.matmul(
                            psq,
                            lhsT=wq_sb[:, kc, mc * 128 : (mc + 1) * 128],
                            rhs=hT[:, kc, 0:NTOK_MACRO],
                            start=(kc == 0),
                            stop=(kc == 1),
                        )
                    nc.scalar.activation(
                        qT[:, mc, :], psq, AF.Identity, bias=bq_sb[:, mc : mc + 1]
                    )
                    psk = ps_big.tile([128, NTOK_MACRO], F32, tag="ps_big")
                    for kc in range(2):
                        nc.tensor.matmul(
                            psk,
                            lhsT=wk_sb[:, kc, mc * 128 : (mc + 1) * 128],
                            rhs=hT[:, kc, 0:NTOK_MACRO],
                            start=(kc == 0),
                            stop=(kc == 1),
                        )
                    nc.scalar.activation(
                        kT[:, mc, 0:NTOK_MACRO],
                        psk,
                        AF.Identity,
                        bias=bk_sb[:, mc : mc + 1],
                    )

                if stage == 3:
                    cvt = otm_pool.tile([128, C + 8], F32, tag="o_tm")
                    nc.any.tensor_copy(cvt[:, 0:C], qT[:, 0, 0:C])
                    nc.any.tensor_copy(cvt[:, 0:C], kT[:, 0, 0:C])
                    nc.sync.dma_start(
                        of[mt * 128 : (mt + 1) * 128, :], cvt[:, 0:C]
                    )
                    continue

                # ---- v (token-major, with ones column) ----
                vaugs = []
                for ck in range(CHUNKS_PER_MACRO):
                    psv = ps_sm.tile([128, C], F32, tag="ps_sm")
                    for w01 in range(2):
                        for kc in range(2):
                            nc.tensor.matmul(
                                psv[w01 * 64 : w01 * 64 + 64, :],
                                lhsT=hT[
                                    :, kc, (ck * 2 + w01) * 49 : (ck * 2 + w01) * 49 + 64
                                ],
                                rhs=wv_sb[:, kc, :],
                                start=(kc == 0),
                                stop=(kc == 1),
                            )
                    va = vaug_pool.tile([128, NH, HD + 1], BF16, tag="va")
                    nc.any.tensor_copy(
                        va[:, :, 0:HD],
                        psv.rearrange("p (h d) -> p h d", h=NH),
                    )
                    nc.vector.memset(va[:, :, HD : HD + 1], 1.0)
                    vaugs.append(va)

                if stage == 4:
                    cvt = otm_pool.tile([128, C + 8], F32, tag="o_tm")
                    nc.any.tensor_copy(
                        cvt[:, 0:C].rearrange("p (h d) -> p h d", h=NH),
                        vaugs[0][:, :, 0:HD],
                    )
                    nc.sync.dma_start(
                        of[mt * 128 : (mt + 1) * 128, :], cvt[:, 0:C]
                    )
                    continue

                # ---- attention per chunk ----
                # PSUM bank rule: concurrent PE row-groups must hit different
                # banks. Scores use one bank per row-group pair (r, r+2 share
                # a slot serially); AV windows rotate through one bank.
                attn_chunks = []
                for ck in range(CHUNKS_PER_MACRO):
                    # scores psum per row-group r: [j(2x64), g*49+i], heads
                    # h = r + 4g live at partitions r*32, ctile g
                    # es block b = 2r+g holds head h = r+4g; log-bias is
                    # preloaded into PSUM (identity matmul) so exp() emits
                    # the finished softmax numerators, pad keys ~ exp(-87)=0
                    es = es_pool.tile([128, 8, 49], BF16, tag="es")
                    for r in range(4):
                        pss = ps_sc.tile(
                            [128, 2 * 49], F32, tag=f"sc{r % 2}",
                            name=f"pss{r}",
                        )
                        nc.tensor.matmul(
                            pss, lhsT=ident, rhs=expb_sb[:, r, :],
                            start=True, stop=True,
                        )
                        for w01 in range(2):
                            t0 = (ck * 2 + w01) * 49
                            for g in range(2):
                                nc.tensor.matmul(
                                    pss[
                                        w01 * 64 : w01 * 64 + 64,
                                        g * 49 : g * 49 + 49,
                                    ],
                                    lhsT=kT[r * 32 : r * 32 + 32, g, t0 : t0 + 64],
                                    rhs=qT[r * 32 : r * 32 + 32, g, t0 : t0 + 49],
                                    start=False,
                                    stop=True,
                                    tile_position=(r * 32, w01 * 64),
                                    skip_group_check=True,
                                )
                        nc.scalar.activation(es[:, 2 * r:2 * r + 2, :], pss,
                                             AF.Exp)

                    if stage == 5:
                        cvt = otm_pool.tile([128, C + 8], F32, tag="o_tm")
                        nc.any.tensor_copy(cvt[:, 0:C], es.rearrange('p a b -> p (a b)')[:, 0:C])
                        nc.sync.dma_start(
                            of[(mt * 4 + ck) * 98 : (mt * 4 + ck) * 98 + 128, :],
                            cvt[:, 0:C],
                        )
                        continue

                    # AV per window: psum[i(49), h*33+d]; K=64 (es zero rows
                    # kill the pad keys). One bank, windows serialized.
                    va = vaugs[ck]
                    for w01 in range(2):
                        jb = w01 * 64
                        psav = ps_av_pool.tile(
                            [64, NH * (HD + 1)], F32, tag="ps_av", name="psav"
                        )
                        for h in range(NH):
                            nc.tensor.matmul(
                                psav[0:49, h * 33 : h * 33 + 33],
                                lhsT=es[jb : jb + 64,
                                        2 * (h % 4) + h // 4, :],
                                rhs=va[jb : jb + 64, h, :],
                                start=True,
                                stop=True,
                                tile_position=(jb, 0),
                            )
                        # normalize + evict (token-major, all rows valid)
                        psav_v = psav.rearrange("p (h e) -> p h e", h=NH)
                        rd = stats_pool.tile([49, NH], F32, tag="rd")
                        nc.vector.reciprocal(rd, psav_v[0:49, :, HD])
                        a_tm = attn_pool.tile([49, NH, HD], BF16, tag="a_tm")
                        nc.vector.tensor_tensor(
                            a_tm,
                            psav_v[0:49, :, 0:HD],
                            rd[:, :, None].to_broadcast([49, NH, HD]),
                            ALU.mult,
                        )
                        attn_chunks.append(a_tm)

                if stage == 5 or stage == 45:
                    continue
                if stage == 6:
                    cvt = otm_pool.tile([128, C + 8], F32, tag="o_tm")
                    nc.any.tensor_copy(
                        cvt[:49, 0:C].rearrange("p (h d) -> p h d", h=NH),
                        attn_chunks[0],
                    )
                    nc.sync.dma_start(
                        of[mt * 128 : mt * 128 + 49, :], cvt[:49, 0:C]
                    )
                    continue

                # ---- transpose attn to channel-major ----
                aT = attnT_pool.tile([128, 2, NTOK_MACRO], BF16, tag="aT")
                for wk_i in range(2 * CHUNKS_PER_MACRO):
                    a_flat = attn_chunks[wk_i].rearrange("p h d -> p (h d)")
                    for kc in range(2):
                        psat = ps_tr.tile([128, 49], BF16, tag="pst")
                        nc.tensor.transpose(
                            psat,
                            a_flat[:, kc * 128 : (kc + 1) * 128],
                            ident[:49, :49],
                        )
                        nc.any.tensor_copy(
                            aT[:, kc, wk_i * 49 : (wk_i + 1) * 49], psat
                        )

                # ---- proj (token-major out) + residual + LN2 + transpose ----
                h2T = h2T_pool.tile([128, 2, NTOK_MACRO], BF16, tag="h2T")
                x2_chunks = []
                for ck in range(CHUNKS_PER_MACRO):
                    psp = ps_sm.tile([98, C], F32, tag="ps_sm")
                    for kc in range(2):
                        nc.tensor.matmul(
                            psp,
                            lhsT=aT[:, kc, ck * 98 : (ck + 1) * 98],
                            rhs=wp_sb[:, kc, :],
                            start=(kc == 0),
                            stop=(kc == 1),
                        )
                    x2 = x2_pool.tile([98, C], F32, tag="x2")
                    nc.vector.tensor_tensor(x2, psp, x_chunks[ck], ALU.add)
                    x2_chunks.append(x2)
                    # LN2
                    st2 = stats_pool.tile([98, 6], F32, tag="st2")
                    nc.vector.bn_stats(st2, x2)
                    mv2 = stats_pool.tile([98, 2], F32, tag="mv2")
                    nc.vector.bn_aggr(mv2, st2)
                    sd2 = stats_pool.tile([98, 1], F32, tag="sd2")
                    nc.scalar.activation(sd2, mv2[:, 1:2], AF.Sqrt, bias=eps_sb[:98])
                    rstd2 = stats_pool.tile([98, 1], F32, tag="rstd2")
                    nc.vector.reciprocal(rstd2, sd2)
                    xh2 = xhat_pool.tile([98, C], BF16, tag="xh2")
                    nc.vector.tensor_scalar(
                        xh2, x2, mv2[:, 0:1], rstd2, ALU.subtract, ALU.mult
                    )
                    for kc in range(2):
                        pst2 = ps_tr.tile([128, 98], BF16, tag="pst")
                        nc.tensor.transpose(
                            pst2, xh2[:, kc * 128 : (kc + 1) * 128], ident[:98, :98]
                        )
                        nc.any.tensor_copy(
                            h2T[:, kc, ck * 98 : (ck + 1) * 98], pst2
                        )

                if stage == 7:
                    for ck in range(CHUNKS_PER_MACRO):
                        g = mt * MACRO_WINS + ck * 2
                        b_i = g // (NWS * NWS)
                        wr = (g % (NWS * NWS)) // NWS
                        wc = g % NWS
                        for w01 in range(2):
                            dst = ow[b_i, wr, :, wc + w01, :, :]
                            nc.sync.dma_start(
                                dst,
                                x2_chunks[ck][w01 * 49 : (w01 + 1) * 49, :],
                            )
                    continue

                # ---- fc1 + gelu (channel-major) ----
                gT = gT_pool.tile([128, 8, NTOK_MACRO], BF16, tag="gT")
                for mc in range(8):
                    psf = ps_big.tile([128, NTOK_MACRO], F32, tag="ps_big")
                    for kc in range(2):
                        nc.tensor.matmul(
                            psf,
                            lhsT=w1_sb[:, kc, mc * 128 : (mc + 1) * 128],
                            rhs=h2T[:, kc, :],
                            start=(kc == 0),
                            stop=(kc == 1),
                        )
                    nc.scalar.activation(
                        gT[:, mc, :], psf, AF.Gelu, bias=b1_sb[:, mc : mc + 1]
                    )

                # ---- fc2 (token-major out) + residual + store ----
                for ck in range(CHUNKS_PER_MACRO):
                    ps2 = ps_sm.tile([98, C], F32, tag="ps_sm")
                    for hc in range(8):
                        nc.tensor.matmul(
                            ps2,
                            lhsT=gT[:, hc, ck * 98 : (ck + 1) * 98],
                            rhs=w2_sb[:, hc, :],
                            start=(hc == 0),
                            stop=(hc == 7),
                        )
                    o_tile = otm_pool.tile([98, C + 8], F32, tag="o_tm")
                    o_tm = o_tile[:, 0:C]
                    nc.vector.tensor_tensor(o_tm, ps2, x2_chunks[ck], ALU.add)
                    g = mt * MACRO_WINS + ck * 2
                    b_i = g // (NWS * NWS)
                    wr = (g % (NWS * NWS)) // NWS
                    wc = g % NWS
                    for w01 in range(2):
                        dst = ow[b_i, wr, :, wc + w01, :, :]
                        nc.sync.dma_start(
                            dst, o_tm[w01 * 49 : (w01 + 1) * 49, :]
                        )

    if split_waits:
        _split_multi_waits(nc)
    return nc


# ------------------------------------------------------------- host wrapper
_PROGRAM_CACHE = {}


def _prep_weights(norm1_g, norm1_b, qkv_w, qkv_b, bias_table, proj_w, proj_b,
                  norm2_g, norm2_b, fc1_w, fc1_b, fc2_w, fc2_b):
    f32 = np.float32
    bf16 = ml_dtypes.bfloat16
    # fold LN1 affine into qkv weights
    wqkv = (norm1_g[:, None] * qkv_w).astype(f32)  # [C, 3C]
    bqkv = (norm1_b @ qkv_w + qkv_b).astype(f32)  # [3C]
    wq = wqkv[:, 0:C] * SCALE
    bq = bqkv[0:C] * SCALE
    wk = wqkv[:, C : 2 * C]
    bk = bqkv[C : 2 * C]
    wv = wqkv[:, 2 * C : 3 * C]
    bv = bqkv[2 * C : 3 * C]
    # fold LN2 affine into fc1
    w1 = (norm2_g[:, None] * fc1_w).astype(f32)  # [C, HID]
    b1 = (norm2_b @ fc1_w + fc1_b).astype(f32)  # [HID]

    def kpart(w):  # [K, O] -> [128, K//128, O]
        k, o = w.shape
        return np.ascontiguousarray(
            w.reshape(k // 128, 128, o).transpose(1, 0, 2)
        )

    arrs = {
        "wq": kpart(wq).astype(bf16),
        "wk": kpart(wk).astype(bf16),
        "wv": kpart(wv).astype(bf16),
        "wp": kpart(proj_w.astype(f32)).astype(bf16),
        "w1": kpart(w1).astype(bf16),
        "w2": kpart(fc2_w.astype(f32)).astype(bf16),
        "bq": np.ascontiguousarray(bq.reshape(2, 128).T).astype(f32),
        "bk": np.ascontiguousarray(bk.reshape(2, 128).T).astype(f32),
        "b1": np.ascontiguousarray(b1.reshape(8, 128).T).astype(f32),
    }
    # log-domain rel-pos bias, PSUM-preloaded: lb[jrow, r, g*49+i] =
    # bias[i, j, h=r+4g]; -87 at pad key rows so exp() zeroes them
    bias_full = bias_table[REL_IDX]  # [i, j, NH]
    lb = np.full((128, 4, 98), -87.0, dtype=f32)
    for h in range(NH):
        r, g = h % 4, h // 4
        bj = bias_full[:, :, h].T.astype(f32)  # [j, i]
        lb[0:49, r, g * 49:(g + 1) * 49] = bj
        lb[64:113, r, g * 49:(g + 1) * 49] = bj
    arrs["expb"] = lb.astype(ml_dtypes.bfloat16)

    # token-major biases (v, proj, fc2) must be zero for this build
    for name, v in (("bv", bv), ("bp", proj_b), ("b2", fc2_b)):
        assert np.abs(v).max() < 1e-30, f"nonzero {name} not supported yet"
    return arrs


def kernel(**inputs):
    x = np.asarray(inputs["x"], dtype=np.float32)
    prep = _prep_weights(
        np.asarray(inputs["norm1_g"], np.float32),
        np.asarray(inputs["norm1_b"], np.float32),
        np.asarray(inputs["qkv_w"], np.float32),
        np.asarray(inputs["qkv_b"], np.float32),
        np.asarray(inputs["bias_table"], np.float32),
        np.asarray(inputs["proj_w"], np.float32),
        np.asarray(inputs["proj_b"], np.float32),
        np.asarray(inputs["norm2_g"], np.float32),
        np.asarray(inputs["norm2_b"], np.float32),
        np.asarray(inputs["fc1_w"], np.float32),
        np.asarray(inputs["fc1_b"], np.float32),
        np.asarray(inputs["fc2_w"], np.float32),
        np.asarray(inputs["fc2_b"], np.float32),
    )

    if "nc" not in _PROGRAM_CACHE:
        _PROGRAM_CACHE["nc"] = build_program()
    nc = _PROGRAM_CACHE["nc"]

    in_maps = []
    for c in range(N_CORES):
        m = dict(prep)
        m["x"] = np.ascontiguousarray(x[c * BL : (c + 1) * BL])
        in_maps.append(m)

    res = run_bass_kernel_spmd(nc, in_maps, core_ids=list(range(N_CORES)))
    out = np.concatenate([res.results[c]["out"] for c in range(N_CORES)], axis=0)
    return out.astype(np.float32)

